# revision 15
# baseline (speedup 1.0000x reference)
"""Trainium2 Bass kernel for multi-head attention (B=4, N=2048, C=768, H=12).

Sharding: 8 cores = 4 batches x 2 sequence-halves. Each core computes K/V for
its batch's full 2048-token sequence (duplicated across the 2 cores sharing a
batch) and Q/attention/proj for its own 1024 query rows. No collectives; the
host gather is pure concatenation. The host passes x[b].T with the core's own
half rolled to the front, so Q-projection always reads columns 0:1024
(attention is permutation-invariant along keys, so rolling K/V is harmless).

v6: all-bf16 datapath (PSUM and the exp input stay fp32). bf16 stationary
operands get separate LDWEIGHTS, so the two 64-deep QK matmuls of a head pair
run concurrently as PE row tiles (0,0)/(64,0). V tiles are 65 columns (64 hd
+ ones row producing the softmax denominator in PSUM), so no memzero is
needed. ScalarE exp (25.2M elems/core at 1 elem/cyc/lane, ~213us busy) is the
pacing engine. The key structure: the QK+exp stream is DECOUPLED from the
AV/PSUM-accumulator constraint by buffering exp tiles in SBUF — every quad
emits score+exp chunks per token block as soon as K/Q land (attention starts
~8us in, and each quad's exps are ready during the previous quad's attention,
so ScalarE never starves at quad boundaries). AV matmuls trail, consuming
buffered exp tiles into the 2 live PSUM accumulator pairs. Normalization is
per head-pair (denominators packed on partitions 0-1 via tiny DMAs, one DVE
reciprocal, GpSimd partition_broadcast from partition 0, DVE multiply), and
the final projection pre-accumulates head pairs 0-4 during quad-2 attention
so only pair 5 + bias trail the last normalize. Startup DMAs are split
across the Sync and Activation HWDGE queues.
"""

import os
import ml_dtypes
import numpy as np

B, N, C = 4, 2048, 768
H, HD = 12, 64
SCALE = HD ** -0.5
P = 128
CT = C // P          # 6 contraction tiles
PAIRS = H // 2       # 6 head pairs
QUADS = H // 4       # 3 head quads
IQ = N // 2          # 1024 query rows per core
JT = N // P          # 16 key tiles
TKB = 512            # token-block width streamed from DRAM
VW = 72              # per-head stride in v_all (65 used: 64 hd + ones)
NCORES = 8

_cache = {}


def _build_bass():
    import concourse.bass as bass
    import concourse.tile as tile
    import concourse.mybir as mybir
    from concourse import bacc
    from concourse.bass import ts, ds
    from contextlib import ExitStack

    f32 = mybir.dt.float32
    bf16 = mybir.dt.bfloat16
    Exp = mybir.ActivationFunctionType.Exp

    nc = bacc.Bacc("TRN2", target_bir_lowering=False, debug=False)

    xt_d = nc.dram_tensor("xt", [C, N], bf16, kind="ExternalInput").ap()
    wq_d = nc.dram_tensor("wq", [C, C], bf16, kind="ExternalInput").ap()
    wk_d = nc.dram_tensor("wk", [C, C], bf16, kind="ExternalInput").ap()
    wv_d = nc.dram_tensor("wv", [C, C], bf16, kind="ExternalInput").ap()
    wp_d = nc.dram_tensor("wp", [C, C], bf16, kind="ExternalInput").ap()
    bb_d = nc.dram_tensor("bb", [P, C], f32, kind="ExternalInput").ap()
    out_d = nc.dram_tensor("out", [IQ, C], f32, kind="ExternalOutput").ap()

    xt_r = xt_d.rearrange("(o p) n -> p o n", p=P)
    wq_r = wq_d.rearrange("(o p) n -> p o n", p=P)
    wk_r = wk_d.rearrange("(o p) n -> p o n", p=P)
    wv_r = wv_d.rearrange("(o p) n -> p o n", p=P)
    wp_r = wp_d.rearrange("(o p) n -> p o n", p=P)
    out_r = out_d.rearrange("(t p) n -> t p n", p=P)

    with tile.TileContext(nc) as tc:
        with ExitStack() as ctx:
            persist = ctx.enter_context(tc.tile_pool(name="persist", bufs=1))
            outT_sb = persist.tile([P, PAIRS, IQ], bf16, name="outT_sb")
            v_all = persist.tile([P, JT, H * VW], bf16, name="v_all")
            v_all_r = v_all.rearrange("p t (h e) -> p t h e", e=VW)
            with nc.allow_low_precision(reason="ones column"):
                nc.vector.tensor_copy(
                    v_all_r[:, :, :, 64],
                    nc.const_aps.tensor(1.0, [P, JT, H], bf16),
                )

            wpool = ctx.enter_context(tc.tile_pool(name="wq", bufs=2))
            wvpool = ctx.enter_context(tc.tile_pool(name="wv", bufs=1))
            kvq = ctx.enter_context(tc.tile_pool(name="kvq", bufs=2))
            xt_pool = ctx.enter_context(tc.tile_pool(name="xtp", bufs=2))
            apsum = ctx.enter_context(
                tc.tile_pool(name="apsum", bufs=2, space="PSUM")
            )
            spsum = ctx.enter_context(
                tc.tile_pool(name="spsum", bufs=2, space="PSUM")
            )
            opsum = ctx.enter_context(
                tc.tile_pool(name="opsum", bufs=2, space="PSUM")
            )
            # deep exp-tile buffer: lets the QK+exp stream run far ahead of
            # the AV consumers (ib1 blocks' exps are fully buffered)
            expt_pool = ctx.enter_context(tc.tile_pool(name="expt", bufs=34))
            nrm_pool = ctx.enter_context(tc.tile_pool(name="nrm", bufs=2))
            poS_pool = ctx.enter_context(tc.tile_pool(name="poSp", bufs=4))
            ppool = ctx.enter_context(tc.tile_pool(name="pw", bufs=1))
            outsb_pool = ctx.enter_context(tc.tile_pool(name="outsb", bufs=2))

            wp_sb = None
            bias_sb = None

            def attn_qk(kT_q, qT_q, tl, ib, jts):
                ets = []
                for jt in jts:
                    ss = spsum.tile([P, 1024], f32, tag="ss", name="ss")
                    nc.tensor.matmul(
                        ss[:, 0:512],
                        kT_q[0:64, tl, ts(jt, P)],
                        qT_q[0:64, tl, ts(ib, 512)],
                        start=True,
                        stop=True,
                    )
                    nc.tensor.matmul(
                        ss[:, 512:1024],
                        kT_q[64:128, tl, ts(jt, P)],
                        qT_q[64:128, tl, ts(ib, 512)],
                        start=True,
                        stop=True,
                    )
                    et = expt_pool.tile([P, 1024], bf16, tag="et", name="et")
                    nc.scalar.activation(et[:], ss[:], Exp, scale=SCALE)
                    ets.append((jt, et))
                return ets

            def attn_av(q, tl, pos, ets):
                t = 2 * q + tl
                for jt, et in ets:
                    for hh in range(2):
                        hg = 2 * t + hh
                        nc.tensor.matmul(
                            pos[hh][0:65, :],
                            v_all_r[:, jt, hg, 0:65],
                            et[:, hh * 512 : (hh + 1) * 512],
                            start=(jt == 0),
                            stop=(jt == JT - 1),
                        )

            def norm_tl(q, tl, ib, pos):
                """Per-pair softmax normalization: outT = po[0:64] / po[64]."""
                t = 2 * q + tl
                dpk = nrm_pool.tile([2, 512], f32, tag="dpk", name="dpk")
                poSs = []
                for hh in range(2):
                    poS = poS_pool.tile([65, 512], f32, tag="poS", name="poS")
                    nc.vector.tensor_copy(poS[:], pos[hh][0:65, :])
                    nc.sync.dma_start(dpk[hh : hh + 1, :], poS[64:65, :])
                    poSs.append(poS)
                rd_q = nrm_pool.tile([2, 512], f32, tag="rd_q", name="rd_q")
                nc.vector.reciprocal(rd_q[:], dpk[:])
                for hh in range(2):
                    if hh == 0:
                        rd_src = rd_q
                    else:
                        # relocate to partition 0: HW partition_broadcast
                        # only sources partition 0 correctly
                        rd_src = nrm_pool.tile([1, 512], f32, tag="rd1", name="rd1")
                        nc.sync.dma_start(rd_src[:], rd_q[1:2, :])
                    rb_sb = nrm_pool.tile([64, 512], f32, tag="rb_sb", name="rb_sb")
                    nc.gpsimd.partition_broadcast(rb_sb[:], rd_src[0:1, :])
                    with nc.allow_low_precision(reason="bf16 out path"):
                        nc.vector.tensor_mul(
                            outT_sb[hh * 64 : (hh + 1) * 64, t, ts(ib, 512)],
                            poSs[hh][0:64, :],
                            rb_sb[:],
                        )

            def final_git_pre(git):
                """Accumulate head pairs 0..4 of the output projection."""
                pps = []
                for n0, n1 in ((0, 512), (512, 768)):
                    pp = apsum.tile([P, 512], f32, tag="aps", name="pp")
                    for t in range(PAIRS - 1):
                        nc.tensor.matmul(
                            pp[:, 0 : n1 - n0],
                            outT_sb[:, t, ds(git * P, P)],
                            wp_sb[:, t, n0:n1],
                            start=(t == 0),
                            stop=False,
                        )
                    pps.append(pp)
                return pps

            def final_git_post(git, pps, tail=False):
                """Last head pair + bias. The out DMA rides the Activation
                HWDGE queue only in the tail (after the last exp) — earlier it
                would block the ACT instruction stream."""
                ob = outsb_pool.tile([P, C], f32, tag="ob", name="ob")
                for (n0, n1), pp in zip(((0, 512), (512, 768)), pps):
                    nc.tensor.matmul(
                        pp[:, 0 : n1 - n0],
                        outT_sb[:, PAIRS - 1, ds(git * P, P)],
                        wp_sb[:, PAIRS - 1, n0:n1],
                        start=False,
                        stop=True,
                    )
                    nc.vector.tensor_add(
                        ob[:, n0:n1], pp[:, 0 : n1 - n0], bias_sb[:, n0:n1]
                    )
                (nc.scalar if tail else nc.sync).dma_start(out_r[git], ob[:])

            for q in range(QUADS):
                # ---- load this quad's weight slices ----
                wk_t = wpool.tile([P, CT, 256], bf16, tag="wk_t")
                wq_t = wpool.tile([P, CT, 256], bf16, tag="wq_t")
                if q == 0:
                    # parallelize the cold-start loads: wk/wq on the
                    # Activation queue, xt/wv on Sync
                    nc.scalar.dma_start(wk_t[:], wk_r[:, :, ts(q, 256)])
                    nc.scalar.dma_start(wq_t[:], wq_r[:, :, ts(q, 256)])
                else:
                    nc.sync.dma_start(wk_t[:], wk_r[:, :, ts(q, 256)])
                    nc.sync.dma_start(wq_t[:], wq_r[:, :, ts(q, 256)])
                if q == 0:
                    wv_t = wvpool.tile([P, CT, 512], bf16, tag="wv_t", name="wv_t")
                    nc.sync.dma_start(wv_t[:], wv_r[:, :, 0:512])
                elif q == 1:
                    wv_t = wvpool.tile([P, CT, 256], bf16, tag="wv_t", name="wv_t")
                    nc.sync.dma_start(wv_t[:], wv_r[:, :, 512:768])
                if q == 2:
                    # stage final-projection weights during quad 2
                    wp_sb = ppool.tile([P, CT, C], bf16, name="wp_sb")
                    nc.sync.dma_start(wp_sb[:], wp_r)
                    bias_sb = ppool.tile([P, C], f32, name="bias_sb")
                    nc.sync.dma_start(bias_sb[:], bb_d)

                kT_q = kvq.tile([P, 2, N], bf16, tag="kT_q")
                qT_q = kvq.tile([P, 2, IQ], bf16, tag="qT_q")

                # exp-tile queues; PSUM accumulator pairs for the ib0 blocks
                # (their AVs drain inside the tb loop, after each V write)
                ets = {(ib, tl): [] for ib in range(2) for tl in range(2)}
                pos0 = {
                    tl: (
                        opsum.tile([P, 512], f32, tag="po", name="po0"),
                        opsum.tile([P, 512], f32, tag="po", name="po1"),
                    )
                    for tl in range(2)
                }

                # ---- projections + score/exp chunks per token block ----
                for tb in range(N // TKB):
                    xt_t = xt_pool.tile([P, CT, TKB], bf16, tag="xt")
                    nc.sync.dma_start(xt_t[:], xt_r[:, :, ts(tb, TKB)])

                    def k_group(tl):
                        ps = apsum.tile([P, TKB], f32, tag="aps", name="ps")
                        for c in range(CT):
                            nc.tensor.matmul(
                                ps[:],
                                wk_t[:, c, ts(tl, P)],
                                xt_t[:, c, :],
                                start=(c == 0),
                                stop=(c == CT - 1),
                            )
                        with nc.allow_low_precision(reason="bf16 k path"):
                            nc.vector.tensor_copy(kT_q[:, tl, ts(tb, TKB)], ps[:])

                    def q_group(tl):
                        ps = apsum.tile([P, TKB], f32, tag="aps", name="ps")
                        for c in range(CT):
                            nc.tensor.matmul(
                                ps[:],
                                wq_t[:, c, ts(tl, P)],
                                xt_t[:, c, :],
                                start=(c == 0),
                                stop=(c == CT - 1),
                            )
                        with nc.allow_low_precision(reason="bf16 q path"):
                            nc.vector.tensor_copy(qT_q[:, tl, ts(tb, TKB)], ps[:])

                    def v_groups():
                        vn = 512 if q == 0 else 256
                        h0 = 0 if q == 0 else 8
                        for tt in range(TKB // P):
                            ps = apsum.tile([P, vn], f32, tag="aps", name="ps")
                            for c in range(CT):
                                nc.tensor.matmul(
                                    ps[:],
                                    xt_t[:, c, ts(tt, P)],
                                    wv_t[:, c, 0:vn],
                                    start=(c == 0),
                                    stop=(c == CT - 1),
                                )
                            gtt = (tb * TKB) // P + tt
                            with nc.allow_low_precision(reason="bf16 v path"):
                                nc.vector.tensor_copy(
                                    v_all_r[:, gtt, h0 : h0 + vn // 64, 0:64],
                                    ps.rearrange("p (h e) -> p h e", e=64),
                                )

                    k_group(0)
                    k_group(1)
                    if tb < IQ // TKB:
                        q_group(0)
                        q_group(1)
                    if q < 2:
                        v_groups()

                    # score/exp chunks for every block whose K/Q slices are
                    # ready (ib0 tracks tb; ib1 lags one block); exp tiles
                    # buffer in SBUF, AV consumption trails
                    for tl in range(2):
                        ets[(0, tl)] += attn_qk(
                            kT_q, qT_q, tl, 0, range(4 * tb, 4 * tb + 4)
                        )
                    if tb >= 1:
                        for tl in range(2):
                            ets[(1, tl)] += attn_qk(
                                kT_q, qT_q, tl, 1,
                                range(4 * (tb - 1), 4 * tb),
                            )
                    # drain AV for the ib0 blocks (emitted after this tb's V
                    # writes — program order defines RAW semantics on v_all)
                    for tl in range(2):
                        attn_av(q, tl, pos0[tl], ets[(0, tl)])
                        ets[(0, tl)] = []

                # ---- finish: ib0 normalize, ib1 blocks, final projection ----
                for tl in range(2):
                    norm_tl(q, tl, 0, pos0[tl])
                if q == 2:
                    pre01 = [(g, final_git_pre(g)) for g in (0, 1)]
                    for g, pps in pre01:
                        final_git_post(g, pps)
                    for g in (2, 3):
                        final_git_post(g, final_git_pre(g))
                pre45 = None
                for tl in range(2):
                    ets[(1, tl)] += attn_qk(kT_q, qT_q, tl, 1, range(12, JT))
                    pos = (
                        opsum.tile([P, 512], f32, tag="po", name="po0"),
                        opsum.tile([P, 512], f32, tag="po", name="po1"),
                    )
                    attn_av(q, tl, pos, ets[(1, tl)])
                    norm_tl(q, tl, 1, pos)
                    if q == 2 and tl == 0:
                        pre45 = [(g, final_git_pre(g)) for g in (4, 5)]
                if q == 2:
                    for g, pps in pre45:
                        final_git_post(g, pps, tail=True)
                    for g in (6, 7):
                        final_git_post(g, final_git_pre(g), tail=True)

    nc.compile()
    return nc


def _get_nc():
    if "nc" not in _cache:
        _cache["nc"] = _build_bass()
    return _cache["nc"]


def _prep_in_maps(x, w_qkv, w_proj, b_proj):
    x = np.asarray(x, np.float32)
    w_qkv = np.asarray(w_qkv, np.float32)
    w_proj = np.asarray(w_proj, np.float32)
    b_proj = np.asarray(b_proj, np.float32)

    bf = ml_dtypes.bfloat16
    wq = np.ascontiguousarray(w_qkv[0:C].T).astype(bf)
    wk = np.ascontiguousarray(w_qkv[C : 2 * C].T).astype(bf)
    wv = np.ascontiguousarray(w_qkv[2 * C : 3 * C].T).astype(bf)
    wp = np.ascontiguousarray(w_proj.T).astype(bf)
    bb = np.ascontiguousarray(np.broadcast_to(b_proj[None, :], (P, C)))

    in_maps = []
    for core in range(NCORES):
        b, half = core // 2, core % 2
        xT = x[b].T  # [C, N]
        mine = xT[:, half * IQ : (half + 1) * IQ]
        other = xT[:, (1 - half) * IQ : (2 - half) * IQ]
        xt = np.ascontiguousarray(np.concatenate([mine, other], axis=1)).astype(bf)
        in_maps.append(
            {"xt": xt, "wq": wq, "wk": wk, "wv": wv, "wp": wp, "bb": bb}
        )
    return in_maps


def run(x, w_qkv, w_proj, b_proj, trace=False):
    from concourse import bass_utils

    nc = _get_nc()
    in_maps = _prep_in_maps(x, w_qkv, w_proj, b_proj)
    br = bass_utils.run_bass_kernel_spmd(
        nc, in_maps, core_ids=list(range(NCORES)), trace=trace
    )
    y = np.empty((B, N, C), np.float32)
    for core in range(NCORES):
        b, half = core // 2, core % 2
        y[b, half * IQ : (half + 1) * IQ, :] = br.results[core]["out"]
    return y, br


def kernel(x, w_qkv, w_proj, b_proj):
    y, _ = run(x, w_qkv, w_proj, b_proj, trace=False)
    return y


# revision 17
# speedup vs baseline: 1.0327x; 1.0327x over previous
"""Trainium2 Bass kernel for multi-head attention (B=4, N=2048, C=768, H=12).

Sharding: 8 cores = 4 batches x 2 sequence-halves. Each core computes K/V for
its batch's full 2048-token sequence (duplicated across the 2 cores sharing a
batch) and Q/attention/proj for its own 1024 query rows. No collectives; the
host gather is pure concatenation. The host passes x[b].T with the core's own
half rolled to the front, so Q-projection always reads columns 0:1024
(attention is permutation-invariant along keys, so rolling K/V is harmless).

v6: all-bf16 datapath (PSUM and the exp input stay fp32). bf16 stationary
operands get separate LDWEIGHTS, so the two 64-deep QK matmuls of a head pair
run concurrently as PE row tiles (0,0)/(64,0). V tiles are 65 columns (64 hd
+ ones row producing the softmax denominator in PSUM), so no memzero is
needed. ScalarE exp (25.2M elems/core at 1 elem/cyc/lane, ~213us busy) is the
pacing engine. The key structure: the QK+exp stream is DECOUPLED from the
AV/PSUM-accumulator constraint by buffering exp tiles in SBUF — every quad
emits score+exp chunks per token block as soon as K/Q land (attention starts
~8us in, and each quad's exps are ready during the previous quad's attention,
so ScalarE never starves at quad boundaries). AV matmuls trail, consuming
buffered exp tiles into the 2 live PSUM accumulator pairs. Normalization is
per head-pair (denominators packed on partitions 0-1 via tiny DMAs, one DVE
reciprocal, GpSimd partition_broadcast from partition 0, DVE multiply), and
the final projection pre-accumulates head pairs 0-4 during quad-2 attention
so only pair 5 + bias trail the last normalize. Startup DMAs are split
across the Sync and Activation HWDGE queues.
"""

import os
import ml_dtypes
import numpy as np

B, N, C = 4, 2048, 768
H, HD = 12, 64
SCALE = HD ** -0.5
P = 128
CT = C // P          # 6 contraction tiles
PAIRS = H // 2       # 6 head pairs
QUADS = H // 4       # 3 head quads
IQ = N // 2          # 1024 query rows per core
JT = N // P          # 16 key tiles
TKB = 512            # token-block width streamed from DRAM
VW = 72              # per-head stride in v_all (65 used: 64 hd + ones)
NCORES = 8

_cache = {}


def _build_bass():
    import concourse.bass as bass
    import concourse.tile as tile
    import concourse.mybir as mybir
    from concourse import bacc
    from concourse.bass import ts, ds
    from contextlib import ExitStack

    f32 = mybir.dt.float32
    bf16 = mybir.dt.bfloat16
    Exp = mybir.ActivationFunctionType.Exp

    nc = bacc.Bacc("TRN2", target_bir_lowering=False, debug=False)

    xt_d = nc.dram_tensor(
        "xt", [N // TKB, P, CT, TKB], bf16, kind="ExternalInput"
    ).ap()
    wq_d = nc.dram_tensor("wq", [C, C], bf16, kind="ExternalInput").ap()
    wk_d = nc.dram_tensor("wk", [C, C], bf16, kind="ExternalInput").ap()
    wv_d = nc.dram_tensor("wv", [C, C], bf16, kind="ExternalInput").ap()
    wp_d = nc.dram_tensor("wp", [C, C], bf16, kind="ExternalInput").ap()
    bb_d = nc.dram_tensor("bb", [P, C], f32, kind="ExternalInput").ap()
    out_d = nc.dram_tensor("out", [IQ, C], f32, kind="ExternalOutput").ap()

    wq_r = wq_d.rearrange("(o p) n -> p o n", p=P)
    wk_r = wk_d.rearrange("(o p) n -> p o n", p=P)
    wv_r = wv_d.rearrange("(o p) n -> p o n", p=P)
    wp_r = wp_d.rearrange("(o p) n -> p o n", p=P)
    out_r = out_d.rearrange("(t p) n -> t p n", p=P)

    with tile.TileContext(nc) as tc:
        with ExitStack() as ctx:
            persist = ctx.enter_context(tc.tile_pool(name="persist", bufs=1))
            outT_sb = persist.tile([P, PAIRS, IQ], bf16, name="outT_sb")
            v_all = persist.tile([P, JT, H * VW], bf16, name="v_all")
            v_all_r = v_all.rearrange("p t (h e) -> p t h e", e=VW)
            with nc.allow_low_precision(reason="ones column"):
                nc.vector.tensor_copy(
                    v_all_r[:, :, :, 64],
                    nc.const_aps.tensor(1.0, [P, JT, H], bf16),
                )

            wpool = ctx.enter_context(tc.tile_pool(name="wq", bufs=2))
            wvpool = ctx.enter_context(tc.tile_pool(name="wv", bufs=1))
            kvq = ctx.enter_context(tc.tile_pool(name="kvq", bufs=2))
            xt_pool = ctx.enter_context(tc.tile_pool(name="xtp", bufs=2))
            apsum = ctx.enter_context(
                tc.tile_pool(name="apsum", bufs=2, space="PSUM")
            )
            spsum = ctx.enter_context(
                tc.tile_pool(name="spsum", bufs=2, space="PSUM")
            )
            opsum = ctx.enter_context(
                tc.tile_pool(name="opsum", bufs=2, space="PSUM")
            )
            # deep exp-tile buffer: lets the QK+exp stream run far ahead of
            # the AV consumers (ib1 blocks' exps are fully buffered)
            expt_pool = ctx.enter_context(tc.tile_pool(name="expt", bufs=34))
            nrm_pool = ctx.enter_context(tc.tile_pool(name="nrm", bufs=2))
            poS_pool = ctx.enter_context(tc.tile_pool(name="poSp", bufs=4))
            ppool = ctx.enter_context(tc.tile_pool(name="pw", bufs=1))
            outsb_pool = ctx.enter_context(tc.tile_pool(name="outsb", bufs=2))

            wp_sb = None
            bias_sb = None

            def attn_qk(kT_q, qT_q, tl, ib, jts):
                ets = []
                for jt in jts:
                    ss = spsum.tile([P, 1024], f32, tag="ss", name="ss")
                    nc.tensor.matmul(
                        ss[:, 0:512],
                        kT_q[0:64, tl, ts(jt, P)],
                        qT_q[0:64, tl, ts(ib, 512)],
                        start=True,
                        stop=True,
                    )
                    nc.tensor.matmul(
                        ss[:, 512:1024],
                        kT_q[64:128, tl, ts(jt, P)],
                        qT_q[64:128, tl, ts(ib, 512)],
                        start=True,
                        stop=True,
                    )
                    et = expt_pool.tile([P, 1024], bf16, tag="et", name="et")
                    nc.scalar.activation(et[:], ss[:], Exp, scale=SCALE)
                    ets.append((jt, et))
                return ets

            def attn_av(q, tl, pos, ets):
                t = 2 * q + tl
                for jt, et in ets:
                    for hh in range(2):
                        hg = 2 * t + hh
                        nc.tensor.matmul(
                            pos[hh][0:65, :],
                            v_all_r[:, jt, hg, 0:65],
                            et[:, hh * 512 : (hh + 1) * 512],
                            start=(jt == 0),
                            stop=(jt == JT - 1),
                        )

            def norm_tl(q, tl, ib, pos):
                """Per-pair softmax normalization: outT = po[0:64] / po[64]."""
                t = 2 * q + tl
                dpk = nrm_pool.tile([2, 512], f32, tag="dpk", name="dpk")
                poSs = []
                for hh in range(2):
                    poS = poS_pool.tile([65, 512], f32, tag="poS", name="poS")
                    nc.vector.tensor_copy(poS[:], pos[hh][0:65, :])
                    nc.sync.dma_start(dpk[hh : hh + 1, :], poS[64:65, :])
                    poSs.append(poS)
                rd_q = nrm_pool.tile([2, 512], f32, tag="rd_q", name="rd_q")
                nc.vector.reciprocal(rd_q[:], dpk[:])
                for hh in range(2):
                    if hh == 0:
                        rd_src = rd_q
                    else:
                        # relocate to partition 0: HW partition_broadcast
                        # only sources partition 0 correctly
                        rd_src = nrm_pool.tile([1, 512], f32, tag="rd1", name="rd1")
                        nc.sync.dma_start(rd_src[:], rd_q[1:2, :])
                    rb_sb = nrm_pool.tile([64, 512], f32, tag="rb_sb", name="rb_sb")
                    nc.gpsimd.partition_broadcast(rb_sb[:], rd_src[0:1, :])
                    with nc.allow_low_precision(reason="bf16 out path"):
                        nc.vector.tensor_mul(
                            outT_sb[hh * 64 : (hh + 1) * 64, t, ts(ib, 512)],
                            poSs[hh][0:64, :],
                            rb_sb[:],
                        )

            def final_git_pre(git):
                """Accumulate head pairs 0..4 of the output projection."""
                pps = []
                for n0, n1 in ((0, 512), (512, 768)):
                    pp = apsum.tile([P, 512], f32, tag="aps", name="pp")
                    for t in range(PAIRS - 1):
                        nc.tensor.matmul(
                            pp[:, 0 : n1 - n0],
                            outT_sb[:, t, ds(git * P, P)],
                            wp_sb[:, t, n0:n1],
                            start=(t == 0),
                            stop=False,
                        )
                    pps.append(pp)
                return pps

            def final_git_post(git, pps, tail=False):
                """Last head pair + bias. The out DMA rides the Activation
                HWDGE queue only in the tail (after the last exp) — earlier it
                would block the ACT instruction stream."""
                ob = outsb_pool.tile([P, C], f32, tag="ob", name="ob")
                for (n0, n1), pp in zip(((0, 512), (512, 768)), pps):
                    nc.tensor.matmul(
                        pp[:, 0 : n1 - n0],
                        outT_sb[:, PAIRS - 1, ds(git * P, P)],
                        wp_sb[:, PAIRS - 1, n0:n1],
                        start=False,
                        stop=True,
                    )
                    nc.vector.tensor_add(
                        ob[:, n0:n1], pp[:, 0 : n1 - n0], bias_sb[:, n0:n1]
                    )
                (nc.scalar if tail else nc.sync).dma_start(out_r[git], ob[:])

            for q in range(QUADS):
                # ---- load this quad's weight slices ----
                wk_t = wpool.tile([P, CT, 256], bf16, tag="wk_t")
                wq_t = wpool.tile([P, CT, 256], bf16, tag="wq_t")
                if q == 0:
                    # parallelize the cold-start loads: wk/wq on the
                    # Activation queue, xt/wv on Sync
                    nc.scalar.dma_start(wk_t[:], wk_r[:, :, ts(q, 256)])
                    nc.scalar.dma_start(wq_t[:], wq_r[:, :, ts(q, 256)])
                else:
                    nc.sync.dma_start(wk_t[:], wk_r[:, :, ts(q, 256)])
                    nc.sync.dma_start(wq_t[:], wq_r[:, :, ts(q, 256)])
                if q == 0:
                    wv_t = wvpool.tile([P, CT, 512], bf16, tag="wv_t", name="wv_t")
                    nc.scalar.dma_start(wv_t[:], wv_r[:, :, 0:512])
                elif q == 1:
                    wv_t = wvpool.tile([P, CT, 256], bf16, tag="wv_t", name="wv_t")
                    nc.sync.dma_start(wv_t[:], wv_r[:, :, 512:768])
                if q == 2:
                    # stage final-projection weights during quad 2
                    wp_sb = ppool.tile([P, CT, C], bf16, name="wp_sb")
                    nc.sync.dma_start(wp_sb[:], wp_r)
                    bias_sb = ppool.tile([P, C], f32, name="bias_sb")
                    nc.sync.dma_start(bias_sb[:], bb_d)

                kT_q = kvq.tile([P, 2, N], bf16, tag="kT_q")
                qT_q = kvq.tile([P, 2, IQ], bf16, tag="qT_q")

                # exp-tile queues; PSUM accumulator pairs for the ib0 blocks
                # (their AVs drain inside the tb loop, after each V write)
                ets = {(ib, tl): [] for ib in range(2) for tl in range(2)}
                pos0 = {
                    tl: (
                        opsum.tile([P, 512], f32, tag="po", name="po0"),
                        opsum.tile([P, 512], f32, tag="po", name="po1"),
                    )
                    for tl in range(2)
                }

                # ---- projections + score/exp chunks per token block ----
                for tb in range(N // TKB):
                    xt_t = xt_pool.tile([P, CT, TKB], bf16, tag="xt")
                    nc.sync.dma_start(xt_t[:], xt_d[tb])

                    def k_group(tl):
                        ps = apsum.tile([P, TKB], f32, tag="aps", name="ps")
                        for c in range(CT):
                            nc.tensor.matmul(
                                ps[:],
                                wk_t[:, c, ts(tl, P)],
                                xt_t[:, c, :],
                                start=(c == 0),
                                stop=(c == CT - 1),
                            )
                        with nc.allow_low_precision(reason="bf16 k path"):
                            nc.vector.tensor_copy(kT_q[:, tl, ts(tb, TKB)], ps[:])

                    def q_group(tl):
                        ps = apsum.tile([P, TKB], f32, tag="aps", name="ps")
                        for c in range(CT):
                            nc.tensor.matmul(
                                ps[:],
                                wq_t[:, c, ts(tl, P)],
                                xt_t[:, c, :],
                                start=(c == 0),
                                stop=(c == CT - 1),
                            )
                        with nc.allow_low_precision(reason="bf16 q path"):
                            nc.vector.tensor_copy(qT_q[:, tl, ts(tb, TKB)], ps[:])

                    def v_groups():
                        vn = 512 if q == 0 else 256
                        h0 = 0 if q == 0 else 8
                        for tt in range(TKB // P):
                            ps = apsum.tile([P, vn], f32, tag="aps", name="ps")
                            for c in range(CT):
                                nc.tensor.matmul(
                                    ps[:],
                                    xt_t[:, c, ts(tt, P)],
                                    wv_t[:, c, 0:vn],
                                    start=(c == 0),
                                    stop=(c == CT - 1),
                                )
                            gtt = (tb * TKB) // P + tt
                            with nc.allow_low_precision(reason="bf16 v path"):
                                nc.vector.tensor_copy(
                                    v_all_r[:, gtt, h0 : h0 + vn // 64, 0:64],
                                    ps.rearrange("p (h e) -> p h e", e=64),
                                )

                    k_group(0)
                    k_group(1)
                    if tb < IQ // TKB:
                        q_group(0)
                        q_group(1)

                    # score/exp chunks for every block whose K/Q slices are
                    # ready (ib0 tracks tb; ib1 lags one block); emitted
                    # BEFORE the V groups so the first exps aren't queued
                    # behind them. Exp tiles buffer in SBUF; AV trails.
                    for tl in range(2):
                        ets[(0, tl)] += attn_qk(
                            kT_q, qT_q, tl, 0, range(4 * tb, 4 * tb + 4)
                        )
                    if tb >= 1:
                        for tl in range(2):
                            ets[(1, tl)] += attn_qk(
                                kT_q, qT_q, tl, 1,
                                range(4 * (tb - 1), 4 * tb),
                            )
                    if q < 2:
                        v_groups()
                    # drain ib0 AVs lagging 2 key-tiles: their exps are long
                    # done, so the PE never stalls on a just-finished ACT
                    # (emitted after this tb's V writes — program order
                    # defines RAW semantics on v_all)
                    for tl in range(2):
                        attn_av(q, tl, pos0[tl], ets[(0, tl)][:-2])
                        ets[(0, tl)] = ets[(0, tl)][-2:]

                # ---- finish: ib0 drain+normalize, ib1 blocks, final proj ----
                for tl in range(2):
                    attn_av(q, tl, pos0[tl], ets[(0, tl)])
                    ets[(0, tl)] = []
                for tl in range(2):
                    norm_tl(q, tl, 0, pos0[tl])
                if q == 2:
                    pre01 = [(g, final_git_pre(g)) for g in (0, 1)]
                    for g, pps in pre01:
                        final_git_post(g, pps)
                    for g in (2, 3):
                        final_git_post(g, final_git_pre(g))
                pre45 = None
                for tl in range(2):
                    ets[(1, tl)] += attn_qk(kT_q, qT_q, tl, 1, range(12, JT))
                    pos = (
                        opsum.tile([P, 512], f32, tag="po", name="po0"),
                        opsum.tile([P, 512], f32, tag="po", name="po1"),
                    )
                    attn_av(q, tl, pos, ets[(1, tl)])
                    norm_tl(q, tl, 1, pos)
                    if q == 2 and tl == 0:
                        pre45 = [(g, final_git_pre(g)) for g in (4, 5)]
                if q == 2:
                    for g, pps in pre45:
                        final_git_post(g, pps, tail=True)
                    for g in (6, 7):
                        final_git_post(g, final_git_pre(g), tail=True)

    nc.compile()
    return nc


def _get_nc():
    if "nc" not in _cache:
        _cache["nc"] = _build_bass()
    return _cache["nc"]


def _prep_in_maps(x, w_qkv, w_proj, b_proj):
    x = np.asarray(x, np.float32)
    w_qkv = np.asarray(w_qkv, np.float32)
    w_proj = np.asarray(w_proj, np.float32)
    b_proj = np.asarray(b_proj, np.float32)

    bf = ml_dtypes.bfloat16
    wq = np.ascontiguousarray(w_qkv[0:C].T).astype(bf)
    wk = np.ascontiguousarray(w_qkv[C : 2 * C].T).astype(bf)
    wv = np.ascontiguousarray(w_qkv[2 * C : 3 * C].T).astype(bf)
    wp = np.ascontiguousarray(w_proj.T).astype(bf)
    bb = np.ascontiguousarray(np.broadcast_to(b_proj[None, :], (P, C)))

    in_maps = []
    for core in range(NCORES):
        b, half = core // 2, core % 2
        xT = x[b].T  # [C, N]
        mine = xT[:, half * IQ : (half + 1) * IQ]
        other = xT[:, (1 - half) * IQ : (2 - half) * IQ]
        xt2 = np.concatenate([mine, other], axis=1)  # [C, N]
        # tiled [tb, p, o, n]: 6KB-contiguous per (tb, partition) DMA runs
        xt = np.ascontiguousarray(
            xt2.reshape(CT, P, N // TKB, TKB).transpose(2, 1, 0, 3)
        ).astype(bf)
        in_maps.append(
            {"xt": xt, "wq": wq, "wk": wk, "wv": wv, "wp": wp, "bb": bb}
        )
    return in_maps


def run(x, w_qkv, w_proj, b_proj, trace=False):
    from concourse import bass_utils

    nc = _get_nc()
    in_maps = _prep_in_maps(x, w_qkv, w_proj, b_proj)
    br = bass_utils.run_bass_kernel_spmd(
        nc, in_maps, core_ids=list(range(NCORES)), trace=trace
    )
    y = np.empty((B, N, C), np.float32)
    for core in range(NCORES):
        b, half = core // 2, core % 2
        y[b, half * IQ : (half + 1) * IQ, :] = br.results[core]["out"]
    return y, br


def kernel(x, w_qkv, w_proj, b_proj):
    y, _ = run(x, w_qkv, w_proj, b_proj, trace=False)
    return y


# revision 18
# speedup vs baseline: 1.0437x; 1.0107x over previous
"""Trainium2 Bass kernel for multi-head attention (B=4, N=2048, C=768, H=12).

Sharding: 8 cores = 4 batches x 2 sequence-halves. Each core computes K/V for
its batch's full 2048-token sequence (duplicated across the 2 cores sharing a
batch) and Q/attention/proj for its own 1024 query rows. No collectives; the
host gather is pure concatenation. The host passes x[b].T with the core's own
half rolled to the front, so Q-projection always reads columns 0:1024
(attention is permutation-invariant along keys, so rolling K/V is harmless).

v6: all-bf16 datapath (PSUM and the exp input stay fp32). bf16 stationary
operands get separate LDWEIGHTS, so the two 64-deep QK matmuls of a head pair
run concurrently as PE row tiles (0,0)/(64,0). V tiles are 65 columns (64 hd
+ ones row producing the softmax denominator in PSUM), so no memzero is
needed. ScalarE exp (25.2M elems/core at 1 elem/cyc/lane, ~213us busy) is the
pacing engine. The key structure: the QK+exp stream is DECOUPLED from the
AV/PSUM-accumulator constraint by buffering exp tiles in SBUF — every quad
emits score+exp chunks per token block as soon as K/Q land (attention starts
~8us in, and each quad's exps are ready during the previous quad's attention,
so ScalarE never starves at quad boundaries). AV matmuls trail, consuming
buffered exp tiles into the 2 live PSUM accumulator pairs. Normalization is
per head-pair (denominators packed on partitions 0-1 via tiny DMAs, one DVE
reciprocal, GpSimd partition_broadcast from partition 0, DVE multiply), and
the final projection pre-accumulates head pairs 0-4 during quad-2 attention
so only pair 5 + bias trail the last normalize. Startup DMAs are split
across the Sync and Activation HWDGE queues.
"""

import os
import ml_dtypes
import numpy as np

B, N, C = 4, 2048, 768
H, HD = 12, 64
SCALE = HD ** -0.5
P = 128
CT = C // P          # 6 contraction tiles
PAIRS = H // 2       # 6 head pairs
QUADS = H // 4       # 3 head quads
IQ = N // 2          # 1024 query rows per core
JT = N // P          # 16 key tiles
TKB = 512            # token-block width streamed from DRAM
VW = 72              # per-head stride in v_all (65 used: 64 hd + ones)
NCORES = 8

_cache = {}


def _build_bass():
    import concourse.bass as bass
    import concourse.tile as tile
    import concourse.mybir as mybir
    from concourse import bacc
    from concourse.bass import ts, ds
    from contextlib import ExitStack

    f32 = mybir.dt.float32
    bf16 = mybir.dt.bfloat16
    Exp = mybir.ActivationFunctionType.Exp

    nc = bacc.Bacc("TRN2", target_bir_lowering=False, debug=False)

    xt_d = nc.dram_tensor(
        "xt", [N // TKB, P, CT, TKB], bf16, kind="ExternalInput"
    ).ap()
    wq_d = nc.dram_tensor("wq", [C, C], bf16, kind="ExternalInput").ap()
    wk_d = nc.dram_tensor("wk", [C, C], bf16, kind="ExternalInput").ap()
    wv_d = nc.dram_tensor("wv", [C, C], bf16, kind="ExternalInput").ap()
    wp_d = nc.dram_tensor("wp", [C, C], bf16, kind="ExternalInput").ap()
    bb_d = nc.dram_tensor("bb", [P, C], f32, kind="ExternalInput").ap()
    out_d = nc.dram_tensor("out", [IQ, C], f32, kind="ExternalOutput").ap()

    wq_r = wq_d.rearrange("(o p) n -> p o n", p=P)
    wk_r = wk_d.rearrange("(o p) n -> p o n", p=P)
    wv_r = wv_d.rearrange("(o p) n -> p o n", p=P)
    wp_r = wp_d.rearrange("(o p) n -> p o n", p=P)
    out_r = out_d.rearrange("(t p) n -> t p n", p=P)

    with tile.TileContext(nc) as tc:
        with ExitStack() as ctx:
            persist = ctx.enter_context(tc.tile_pool(name="persist", bufs=1))
            outT_sb = persist.tile([P, PAIRS, IQ], bf16, name="outT_sb")
            v_all = persist.tile([P, JT, H * VW], bf16, name="v_all")
            v_all_r = v_all.rearrange("p t (h e) -> p t h e", e=VW)
            with nc.allow_low_precision(reason="ones column"):
                nc.vector.tensor_copy(
                    v_all_r[:, :, :, 64],
                    nc.const_aps.tensor(1.0, [P, JT, H], bf16),
                )

            wpool = ctx.enter_context(tc.tile_pool(name="wq", bufs=2))
            wvpool = ctx.enter_context(tc.tile_pool(name="wv", bufs=1))
            kvq = ctx.enter_context(tc.tile_pool(name="kvq", bufs=2))
            xt_pool = ctx.enter_context(tc.tile_pool(name="xtp", bufs=2))
            apsum = ctx.enter_context(
                tc.tile_pool(name="apsum", bufs=2, space="PSUM")
            )
            spsum = ctx.enter_context(
                tc.tile_pool(name="spsum", bufs=2, space="PSUM")
            )
            opsum = ctx.enter_context(
                tc.tile_pool(name="opsum", bufs=2, space="PSUM")
            )
            # deep exp-tile buffer: lets the QK+exp stream run far ahead of
            # the AV consumers (ib1 blocks' exps are fully buffered)
            expt_pool = ctx.enter_context(tc.tile_pool(name="expt", bufs=30))
            nrm_pool = ctx.enter_context(tc.tile_pool(name="nrm", bufs=2))
            poS_pool = ctx.enter_context(tc.tile_pool(name="poSp", bufs=4))
            ppool = ctx.enter_context(tc.tile_pool(name="pw", bufs=1))
            outsb_pool = ctx.enter_context(tc.tile_pool(name="outsb", bufs=2))

            wp_sb = None
            bias_sb = None

            def attn_qk(kT_q, qT_q, tl, ib, jts):
                ets = []
                for jt in jts:
                    ss = spsum.tile([P, 1024], f32, tag="ss", name="ss")
                    nc.tensor.matmul(
                        ss[:, 0:512],
                        kT_q[0:64, tl, ts(jt, P)],
                        qT_q[0:64, tl, ts(ib, 512)],
                        start=True,
                        stop=True,
                    )
                    nc.tensor.matmul(
                        ss[:, 512:1024],
                        kT_q[64:128, tl, ts(jt, P)],
                        qT_q[64:128, tl, ts(ib, 512)],
                        start=True,
                        stop=True,
                    )
                    et = expt_pool.tile([P, 1024], bf16, tag="et", name="et")
                    nc.scalar.activation(et[:], ss[:], Exp, scale=SCALE)
                    ets.append((jt, et))
                return ets

            def attn_av(q, tl, pos, ets):
                t = 2 * q + tl
                for jt, et in ets:
                    for hh in range(2):
                        hg = 2 * t + hh
                        nc.tensor.matmul(
                            pos[hh][0:65, :],
                            v_all_r[:, jt, hg, 0:65],
                            et[:, hh * 512 : (hh + 1) * 512],
                            start=(jt == 0),
                            stop=(jt == JT - 1),
                        )

            def norm_tl(q, tl, ib, pos):
                """Per-pair softmax normalization: outT = po[0:64] / po[64]."""
                t = 2 * q + tl
                dpk = nrm_pool.tile([2, 512], f32, tag="dpk", name="dpk")
                poSs = []
                for hh in range(2):
                    poS = poS_pool.tile([65, 512], f32, tag="poS", name="poS")
                    nc.vector.tensor_copy(poS[:], pos[hh][0:65, :])
                    nc.sync.dma_start(dpk[hh : hh + 1, :], poS[64:65, :])
                    poSs.append(poS)
                rd_q = nrm_pool.tile([2, 512], f32, tag="rd_q", name="rd_q")
                nc.vector.reciprocal(rd_q[:], dpk[:])
                for hh in range(2):
                    if hh == 0:
                        rd_src = rd_q
                    else:
                        # relocate to partition 0: HW partition_broadcast
                        # only sources partition 0 correctly
                        rd_src = nrm_pool.tile([1, 512], f32, tag="rd1", name="rd1")
                        nc.sync.dma_start(rd_src[:], rd_q[1:2, :])
                    rb_sb = nrm_pool.tile([64, 512], f32, tag="rb_sb", name="rb_sb")
                    nc.gpsimd.partition_broadcast(rb_sb[:], rd_src[0:1, :])
                    with nc.allow_low_precision(reason="bf16 out path"):
                        nc.vector.tensor_mul(
                            outT_sb[hh * 64 : (hh + 1) * 64, t, ts(ib, 512)],
                            poSs[hh][0:64, :],
                            rb_sb[:],
                        )

            def final_git_pre(git):
                """Accumulate head pairs 0..4 of the output projection."""
                pps = []
                for n0, n1 in ((0, 512), (512, 768)):
                    pp = apsum.tile([P, 512], f32, tag="aps", name="pp")
                    for t in range(PAIRS - 1):
                        nc.tensor.matmul(
                            pp[:, 0 : n1 - n0],
                            outT_sb[:, t, ds(git * P, P)],
                            wp_sb[:, t, n0:n1],
                            start=(t == 0),
                            stop=False,
                        )
                    pps.append(pp)
                return pps

            def final_git_post(git, pps, tail=False):
                """Last head pair + bias. The out DMA rides the Activation
                HWDGE queue only in the tail (after the last exp) — earlier it
                would block the ACT instruction stream."""
                ob = outsb_pool.tile([P, C], f32, tag="ob", name="ob")
                for (n0, n1), pp in zip(((0, 512), (512, 768)), pps):
                    nc.tensor.matmul(
                        pp[:, 0 : n1 - n0],
                        outT_sb[:, PAIRS - 1, ds(git * P, P)],
                        wp_sb[:, PAIRS - 1, n0:n1],
                        start=False,
                        stop=True,
                    )
                    nc.vector.tensor_add(
                        ob[:, n0:n1], pp[:, 0 : n1 - n0], bias_sb[:, n0:n1]
                    )
                (nc.scalar if tail else nc.sync).dma_start(out_r[git], ob[:])

            for q in range(QUADS):
                # ---- load this quad's weight slices ----
                wk_t = wpool.tile([P, CT, 256], bf16, tag="wk_t")
                wq_t = wpool.tile([P, CT, 256], bf16, tag="wq_t")
                if q == 0:
                    # parallelize the cold-start loads: wk/wq on the
                    # Activation queue, xt/wv on Sync
                    nc.scalar.dma_start(wk_t[:], wk_r[:, :, ts(q, 256)])
                    nc.scalar.dma_start(wq_t[:], wq_r[:, :, ts(q, 256)])
                else:
                    nc.sync.dma_start(wk_t[:], wk_r[:, :, ts(q, 256)])
                    nc.sync.dma_start(wq_t[:], wq_r[:, :, ts(q, 256)])
                if q == 0:
                    wv_t = wvpool.tile([P, CT, 512], bf16, tag="wv_t", name="wv_t")
                    nc.scalar.dma_start(wv_t[:], wv_r[:, :, 0:512])
                elif q == 1:
                    wv_t = wvpool.tile([P, CT, 256], bf16, tag="wv_t", name="wv_t")
                    nc.sync.dma_start(wv_t[:], wv_r[:, :, 512:768])
                if q == 2:
                    # stage final-projection weights during quad 2
                    wp_sb = ppool.tile([P, CT, C], bf16, name="wp_sb")
                    nc.sync.dma_start(wp_sb[:], wp_r)
                    bias_sb = ppool.tile([P, C], f32, name="bias_sb")
                    nc.sync.dma_start(bias_sb[:], bb_d)

                kT_q = kvq.tile([P, 2, N], bf16, tag="kT_q")
                qT_q = kvq.tile([P, 2, IQ], bf16, tag="qT_q")

                # exp-tile queues; PSUM accumulator pairs for the ib0 blocks
                # (their AVs drain inside the tb loop, after each V write)
                ets = {(ib, tl): [] for ib in range(2) for tl in range(2)}
                # only ONE po pair exists at a time (opsum = 2 slots), so
                # exactly one block accumulates in-loop: (ib0, tl0)
                pos0 = (
                    opsum.tile([P, 512], f32, tag="po", name="po0"),
                    opsum.tile([P, 512], f32, tag="po", name="po1"),
                )

                # ---- projections + score/exp chunks per token block ----
                for tb in range(N // TKB):
                    xt_t = xt_pool.tile([P, CT, TKB], bf16, tag="xt")
                    nc.sync.dma_start(xt_t[:], xt_d[tb])

                    def k_group(tl):
                        ps = apsum.tile([P, TKB], f32, tag="aps", name="ps")
                        for c in range(CT):
                            nc.tensor.matmul(
                                ps[:],
                                wk_t[:, c, ts(tl, P)],
                                xt_t[:, c, :],
                                start=(c == 0),
                                stop=(c == CT - 1),
                            )
                        with nc.allow_low_precision(reason="bf16 k path"):
                            nc.vector.tensor_copy(kT_q[:, tl, ts(tb, TKB)], ps[:])

                    def q_group(tl):
                        ps = apsum.tile([P, TKB], f32, tag="aps", name="ps")
                        for c in range(CT):
                            nc.tensor.matmul(
                                ps[:],
                                wq_t[:, c, ts(tl, P)],
                                xt_t[:, c, :],
                                start=(c == 0),
                                stop=(c == CT - 1),
                            )
                        with nc.allow_low_precision(reason="bf16 q path"):
                            nc.vector.tensor_copy(qT_q[:, tl, ts(tb, TKB)], ps[:])

                    def v_groups():
                        vn = 512 if q == 0 else 256
                        h0 = 0 if q == 0 else 8
                        for tt in range(TKB // P):
                            ps = apsum.tile([P, vn], f32, tag="aps", name="ps")
                            for c in range(CT):
                                nc.tensor.matmul(
                                    ps[:],
                                    xt_t[:, c, ts(tt, P)],
                                    wv_t[:, c, 0:vn],
                                    start=(c == 0),
                                    stop=(c == CT - 1),
                                )
                            gtt = (tb * TKB) // P + tt
                            with nc.allow_low_precision(reason="bf16 v path"):
                                nc.vector.tensor_copy(
                                    v_all_r[:, gtt, h0 : h0 + vn // 64, 0:64],
                                    ps.rearrange("p (h e) -> p h e", e=64),
                                )

                    k_group(0)
                    k_group(1)
                    if tb < IQ // TKB:
                        q_group(0)
                        q_group(1)

                    # score/exp chunks for the ib0 blocks (ib1's exps are
                    # produced post-loop: they bridge ScalarE across the
                    # quad boundary while AV backlogs and the next quad's
                    # projections occupy the PE). Emitted BEFORE the V
                    # groups so the first exps aren't queued behind them.
                    for tl in range(2):
                        ets[(0, tl)] += attn_qk(
                            kT_q, qT_q, tl, 0, range(4 * tb, 4 * tb + 4)
                        )
                    if q < 2:
                        v_groups()
                    # drain (ib0, tl0) AVs lagging 2 key-tiles: their exps
                    # are long done so the PE never stalls on a fresh ACT
                    # (emitted after this tb's V writes — program order
                    # defines RAW semantics on v_all)
                    attn_av(q, 0, pos0, ets[(0, 0)][:-2])
                    ets[(0, 0)] = ets[(0, 0)][-2:]

                # ---- finish: sequential blocks, one po pair at a time ----
                attn_av(q, 0, pos0, ets[(0, 0)])
                ets[(0, 0)] = []
                norm_tl(q, 0, 0, pos0)
                # (ib0, tl1): exps already buffered
                pos = (
                    opsum.tile([P, 512], f32, tag="po", name="po0"),
                    opsum.tile([P, 512], f32, tag="po", name="po1"),
                )
                attn_av(q, 1, pos, ets[(0, 1)])
                norm_tl(q, 1, 0, pos)
                if q == 2:
                    pre01 = [(g, final_git_pre(g)) for g in (0, 1)]
                    for g, pps in pre01:
                        final_git_post(g, pps)
                    for g in (2, 3):
                        final_git_post(g, final_git_pre(g))
                # ib1: score/exp emitted now — it keeps ScalarE busy across
                # the quad boundary while the PE runs AV backlogs and the
                # next quad's projections
                pre45 = None
                ets[(1, 0)] += attn_qk(kT_q, qT_q, 0, 1, range(JT))
                ets[(1, 1)] += attn_qk(kT_q, qT_q, 1, 1, range(0, 8))
                for tl in range(2):
                    if tl == 1:
                        ets[(1, 1)] += attn_qk(kT_q, qT_q, 1, 1, range(8, JT))
                    pos = (
                        opsum.tile([P, 512], f32, tag="po", name="po0"),
                        opsum.tile([P, 512], f32, tag="po", name="po1"),
                    )
                    attn_av(q, tl, pos, ets[(1, tl)])
                    norm_tl(q, tl, 1, pos)
                    if q == 2 and tl == 0:
                        pre45 = [(g, final_git_pre(g)) for g in (4, 5)]
                if q == 2:
                    for g, pps in pre45:
                        final_git_post(g, pps, tail=True)
                    for g in (6, 7):
                        final_git_post(g, final_git_pre(g), tail=True)

    nc.compile()
    return nc


def _get_nc():
    if "nc" not in _cache:
        _cache["nc"] = _build_bass()
    return _cache["nc"]


def _prep_in_maps(x, w_qkv, w_proj, b_proj):
    x = np.asarray(x, np.float32)
    w_qkv = np.asarray(w_qkv, np.float32)
    w_proj = np.asarray(w_proj, np.float32)
    b_proj = np.asarray(b_proj, np.float32)

    bf = ml_dtypes.bfloat16
    wq = np.ascontiguousarray(w_qkv[0:C].T).astype(bf)
    wk = np.ascontiguousarray(w_qkv[C : 2 * C].T).astype(bf)
    wv = np.ascontiguousarray(w_qkv[2 * C : 3 * C].T).astype(bf)
    wp = np.ascontiguousarray(w_proj.T).astype(bf)
    bb = np.ascontiguousarray(np.broadcast_to(b_proj[None, :], (P, C)))

    in_maps = []
    for core in range(NCORES):
        b, half = core // 2, core % 2
        xT = x[b].T  # [C, N]
        mine = xT[:, half * IQ : (half + 1) * IQ]
        other = xT[:, (1 - half) * IQ : (2 - half) * IQ]
        xt2 = np.concatenate([mine, other], axis=1)  # [C, N]
        # tiled [tb, p, o, n]: 6KB-contiguous per (tb, partition) DMA runs
        xt = np.ascontiguousarray(
            xt2.reshape(CT, P, N // TKB, TKB).transpose(2, 1, 0, 3)
        ).astype(bf)
        in_maps.append(
            {"xt": xt, "wq": wq, "wk": wk, "wv": wv, "wp": wp, "bb": bb}
        )
    return in_maps


def run(x, w_qkv, w_proj, b_proj, trace=False):
    from concourse import bass_utils

    nc = _get_nc()
    in_maps = _prep_in_maps(x, w_qkv, w_proj, b_proj)
    br = bass_utils.run_bass_kernel_spmd(
        nc, in_maps, core_ids=list(range(NCORES)), trace=trace
    )
    y = np.empty((B, N, C), np.float32)
    for core in range(NCORES):
        b, half = core // 2, core % 2
        y[b, half * IQ : (half + 1) * IQ, :] = br.results[core]["out"]
    return y, br


def kernel(x, w_qkv, w_proj, b_proj):
    y, _ = run(x, w_qkv, w_proj, b_proj, trace=False)
    return y


# revision 19
# speedup vs baseline: 1.0568x; 1.0125x over previous
"""Trainium2 Bass kernel for multi-head attention (B=4, N=2048, C=768, H=12).

Sharding: 8 cores = 4 batches x 2 sequence-halves. Each core computes K/V for
its batch's full 2048-token sequence (duplicated across the 2 cores sharing a
batch) and Q/attention/proj for its own 1024 query rows. No collectives; the
host gather is pure concatenation. The host passes x[b].T with the core's own
half rolled to the front, so Q-projection always reads columns 0:1024
(attention is permutation-invariant along keys, so rolling K/V is harmless).

v6: all-bf16 datapath (PSUM and the exp input stay fp32). bf16 stationary
operands get separate LDWEIGHTS, so the two 64-deep QK matmuls of a head pair
run concurrently as PE row tiles (0,0)/(64,0). V tiles are 65 columns (64 hd
+ ones row producing the softmax denominator in PSUM), so no memzero is
needed. ScalarE exp (25.2M elems/core at 1 elem/cyc/lane, ~213us busy) is the
pacing engine. The key structure: the QK+exp stream is DECOUPLED from the
AV/PSUM-accumulator constraint by buffering exp tiles in SBUF — every quad
emits score+exp chunks per token block as soon as K/Q land (attention starts
~8us in, and each quad's exps are ready during the previous quad's attention,
so ScalarE never starves at quad boundaries). AV matmuls trail, consuming
buffered exp tiles into the 2 live PSUM accumulator pairs. Normalization is
per head-pair (denominators packed on partitions 0-1 via tiny DMAs, one DVE
reciprocal, GpSimd partition_broadcast from partition 0, DVE multiply), and
the final projection pre-accumulates head pairs 0-4 during quad-2 attention
so only pair 5 + bias trail the last normalize. Startup DMAs are split
across the Sync and Activation HWDGE queues.
"""

import os
import ml_dtypes
import numpy as np

B, N, C = 4, 2048, 768
H, HD = 12, 64
SCALE = HD ** -0.5
P = 128
CT = C // P          # 6 contraction tiles
PAIRS = H // 2       # 6 head pairs
QUADS = H // 4       # 3 head quads
IQ = N // 2          # 1024 query rows per core
JT = N // P          # 16 key tiles
TKB = 512            # token-block width streamed from DRAM
VW = 72              # per-head stride in v_all (65 used: 64 hd + ones)
NCORES = 8

_cache = {}


def _build_bass():
    import concourse.bass as bass
    import concourse.tile as tile
    import concourse.mybir as mybir
    from concourse import bacc
    from concourse.bass import ts, ds
    from contextlib import ExitStack

    f32 = mybir.dt.float32
    bf16 = mybir.dt.bfloat16
    Exp = mybir.ActivationFunctionType.Exp

    nc = bacc.Bacc("TRN2", target_bir_lowering=False, debug=False)

    xt_d = nc.dram_tensor(
        "xt", [N // TKB, P, CT, TKB], bf16, kind="ExternalInput"
    ).ap()
    wq_d = nc.dram_tensor("wq", [C, C], bf16, kind="ExternalInput").ap()
    wk_d = nc.dram_tensor("wk", [C, C], bf16, kind="ExternalInput").ap()
    wv_d = nc.dram_tensor("wv", [C, C], bf16, kind="ExternalInput").ap()
    wp_d = nc.dram_tensor("wp", [C, C], bf16, kind="ExternalInput").ap()
    bb_d = nc.dram_tensor("bb", [P, C], f32, kind="ExternalInput").ap()
    out_d = nc.dram_tensor("out", [IQ, C], f32, kind="ExternalOutput").ap()

    wq_r = wq_d.rearrange("(o p) n -> p o n", p=P)
    wk_r = wk_d.rearrange("(o p) n -> p o n", p=P)
    wv_r = wv_d.rearrange("(o p) n -> p o n", p=P)
    wp_r = wp_d.rearrange("(o p) n -> p o n", p=P)
    out_r = out_d.rearrange("(t p) n -> t p n", p=P)

    with tile.TileContext(nc) as tc:
        with ExitStack() as ctx:
            persist = ctx.enter_context(tc.tile_pool(name="persist", bufs=1))
            outT_sb = persist.tile([P, PAIRS, IQ], bf16, name="outT_sb")
            v_all = persist.tile([P, JT, H * VW], bf16, name="v_all")
            v_all_r = v_all.rearrange("p t (h e) -> p t h e", e=VW)
            with nc.allow_low_precision(reason="ones column"):
                nc.vector.tensor_copy(
                    v_all_r[:, :, :, 64],
                    nc.const_aps.tensor(1.0, [P, JT, H], bf16),
                )

            wpool = ctx.enter_context(tc.tile_pool(name="wq", bufs=2))
            wvpool = ctx.enter_context(tc.tile_pool(name="wv", bufs=1))
            kvq = ctx.enter_context(tc.tile_pool(name="kvq", bufs=2))
            xt_pool = ctx.enter_context(tc.tile_pool(name="xtp", bufs=2))
            apsum = ctx.enter_context(
                tc.tile_pool(name="apsum", bufs=2, space="PSUM")
            )
            spsum = ctx.enter_context(
                tc.tile_pool(name="spsum", bufs=2, space="PSUM")
            )
            opsum = ctx.enter_context(
                tc.tile_pool(name="opsum", bufs=2, space="PSUM")
            )
            # deep exp-tile buffer: lets the QK+exp stream run far ahead of
            # the AV consumers (ib1 blocks' exps are fully buffered)
            expt_pool = ctx.enter_context(tc.tile_pool(name="expt", bufs=32))
            nrm_pool = ctx.enter_context(tc.tile_pool(name="nrm", bufs=2))
            poS_pool = ctx.enter_context(tc.tile_pool(name="poSp", bufs=4))
            ppool = ctx.enter_context(tc.tile_pool(name="pw", bufs=1))
            outsb_pool = ctx.enter_context(tc.tile_pool(name="outsb", bufs=2))

            wp_sb = None
            bias_sb = None

            def attn_qk(kT_q, qT_q, tl, ib, jts):
                ets = []
                for jt in jts:
                    ss = spsum.tile([P, 1024], f32, tag="ss", name="ss")
                    nc.tensor.matmul(
                        ss[:, 0:512],
                        kT_q[0:64, tl, ts(jt, P)],
                        qT_q[0:64, tl, ts(ib, 512)],
                        start=True,
                        stop=True,
                    )
                    nc.tensor.matmul(
                        ss[:, 512:1024],
                        kT_q[64:128, tl, ts(jt, P)],
                        qT_q[64:128, tl, ts(ib, 512)],
                        start=True,
                        stop=True,
                    )
                    et = expt_pool.tile([P, 1024], bf16, tag="et", name="et")
                    nc.scalar.activation(et[:], ss[:], Exp, scale=SCALE)
                    ets.append((jt, et))
                return ets

            def attn_av(q, tl, pos, ets):
                t = 2 * q + tl
                for jt, et in ets:
                    for hh in range(2):
                        hg = 2 * t + hh
                        nc.tensor.matmul(
                            pos[hh][0:65, :],
                            v_all_r[:, jt, hg, 0:65],
                            et[:, hh * 512 : (hh + 1) * 512],
                            start=(jt == 0),
                            stop=(jt == JT - 1),
                        )

            def norm_tl(q, tl, ib, pos):
                """Per-pair softmax normalization: outT = po[0:64] / po[64]."""
                t = 2 * q + tl
                dpk = nrm_pool.tile([2, 512], f32, tag="dpk", name="dpk")
                poSs = []
                for hh in range(2):
                    poS = poS_pool.tile([65, 512], f32, tag="poS", name="poS")
                    nc.vector.tensor_copy(poS[:], pos[hh][0:65, :])
                    nc.sync.dma_start(dpk[hh : hh + 1, :], poS[64:65, :])
                    poSs.append(poS)
                rd_q = nrm_pool.tile([2, 512], f32, tag="rd_q", name="rd_q")
                nc.vector.reciprocal(rd_q[:], dpk[:])
                for hh in range(2):
                    if hh == 0:
                        rd_src = rd_q
                    else:
                        # relocate to partition 0: HW partition_broadcast
                        # only sources partition 0 correctly
                        rd_src = nrm_pool.tile([1, 512], f32, tag="rd1", name="rd1")
                        nc.sync.dma_start(rd_src[:], rd_q[1:2, :])
                    rb_sb = nrm_pool.tile([64, 512], f32, tag="rb_sb", name="rb_sb")
                    nc.gpsimd.partition_broadcast(rb_sb[:], rd_src[0:1, :])
                    with nc.allow_low_precision(reason="bf16 out path"):
                        nc.vector.tensor_mul(
                            outT_sb[hh * 64 : (hh + 1) * 64, t, ts(ib, 512)],
                            poSs[hh][0:64, :],
                            rb_sb[:],
                        )

            def final_git_pre(git):
                """Accumulate head pairs 0..4 of the output projection."""
                pps = []
                for n0, n1 in ((0, 512), (512, 768)):
                    pp = apsum.tile([P, 512], f32, tag="aps", name="pp")
                    for t in range(PAIRS - 1):
                        nc.tensor.matmul(
                            pp[:, 0 : n1 - n0],
                            outT_sb[:, t, ds(git * P, P)],
                            wp_sb[:, t, n0:n1],
                            start=(t == 0),
                            stop=False,
                        )
                    pps.append(pp)
                return pps

            def final_git_post(git, pps, tail=False):
                """Last head pair + bias. The out DMA rides the Activation
                HWDGE queue only in the tail (after the last exp) — earlier it
                would block the ACT instruction stream."""
                ob = outsb_pool.tile([P, C], f32, tag="ob", name="ob")
                for (n0, n1), pp in zip(((0, 512), (512, 768)), pps):
                    nc.tensor.matmul(
                        pp[:, 0 : n1 - n0],
                        outT_sb[:, PAIRS - 1, ds(git * P, P)],
                        wp_sb[:, PAIRS - 1, n0:n1],
                        start=False,
                        stop=True,
                    )
                    nc.vector.tensor_add(
                        ob[:, n0:n1], pp[:, 0 : n1 - n0], bias_sb[:, n0:n1]
                    )
                (nc.scalar if tail else nc.sync).dma_start(out_r[git], ob[:])

            for q in range(QUADS):
                # ---- load this quad's weight slices ----
                wk_t = wpool.tile([P, CT, 256], bf16, tag="wk_t")
                wq_t = wpool.tile([P, CT, 256], bf16, tag="wq_t")
                if q == 0:
                    # parallelize the cold-start loads: wk/wq on the
                    # Activation queue, xt/wv on Sync
                    nc.scalar.dma_start(wk_t[:], wk_r[:, :, ts(q, 256)])
                    nc.scalar.dma_start(wq_t[:], wq_r[:, :, ts(q, 256)])
                else:
                    nc.sync.dma_start(wk_t[:], wk_r[:, :, ts(q, 256)])
                    nc.sync.dma_start(wq_t[:], wq_r[:, :, ts(q, 256)])
                if q == 0:
                    wv_t = wvpool.tile([P, CT, 512], bf16, tag="wv_t", name="wv_t")
                    nc.scalar.dma_start(wv_t[:], wv_r[:, :, 0:512])
                elif q == 1:
                    wv_t = wvpool.tile([P, CT, 256], bf16, tag="wv_t", name="wv_t")
                    nc.sync.dma_start(wv_t[:], wv_r[:, :, 512:768])
                if q == 2:
                    # stage final-projection weights during quad 2
                    wp_sb = ppool.tile([P, CT, C], bf16, name="wp_sb")
                    nc.sync.dma_start(wp_sb[:], wp_r)
                    bias_sb = ppool.tile([P, C], f32, name="bias_sb")
                    nc.sync.dma_start(bias_sb[:], bb_d)

                kT_q = kvq.tile([P, 2, N], bf16, tag="kT_q")
                qT_q = kvq.tile([P, 2, IQ], bf16, tag="qT_q")

                # exp-tile queues; PSUM accumulator pairs for the ib0 blocks
                # (their AVs drain inside the tb loop, after each V write)
                ets = {(ib, tl): [] for ib in range(2) for tl in range(2)}
                # only ONE po pair exists at a time (opsum = 2 slots), so
                # exactly one block accumulates in-loop: (ib0, tl0)
                pos0 = (
                    opsum.tile([P, 512], f32, tag="po", name="po0"),
                    opsum.tile([P, 512], f32, tag="po", name="po1"),
                )

                # ---- projections + score/exp chunks per token block ----
                for tb in range(N // TKB):
                    xt_t = xt_pool.tile([P, CT, TKB], bf16, tag="xt")
                    nc.sync.dma_start(xt_t[:], xt_d[tb])

                    def k_group(tl):
                        ps = apsum.tile([P, TKB], f32, tag="aps", name="ps")
                        for c in range(CT):
                            nc.tensor.matmul(
                                ps[:],
                                wk_t[:, c, ts(tl, P)],
                                xt_t[:, c, :],
                                start=(c == 0),
                                stop=(c == CT - 1),
                            )
                        with nc.allow_low_precision(reason="bf16 k path"):
                            nc.vector.tensor_copy(kT_q[:, tl, ts(tb, TKB)], ps[:])

                    def q_group(tl):
                        ps = apsum.tile([P, TKB], f32, tag="aps", name="ps")
                        for c in range(CT):
                            nc.tensor.matmul(
                                ps[:],
                                wq_t[:, c, ts(tl, P)],
                                xt_t[:, c, :],
                                start=(c == 0),
                                stop=(c == CT - 1),
                            )
                        with nc.allow_low_precision(reason="bf16 q path"):
                            nc.vector.tensor_copy(qT_q[:, tl, ts(tb, TKB)], ps[:])

                    def v_groups():
                        vn = 512 if q == 0 else 256
                        h0 = 0 if q == 0 else 8
                        for tt in range(TKB // P):
                            ps = apsum.tile([P, vn], f32, tag="aps", name="ps")
                            for c in range(CT):
                                nc.tensor.matmul(
                                    ps[:],
                                    xt_t[:, c, ts(tt, P)],
                                    wv_t[:, c, 0:vn],
                                    start=(c == 0),
                                    stop=(c == CT - 1),
                                )
                            gtt = (tb * TKB) // P + tt
                            with nc.allow_low_precision(reason="bf16 v path"):
                                nc.vector.tensor_copy(
                                    v_all_r[:, gtt, h0 : h0 + vn // 64, 0:64],
                                    ps.rearrange("p (h e) -> p h e", e=64),
                                )

                    k_group(0)
                    k_group(1)
                    if tb < IQ // TKB:
                        q_group(0)
                        q_group(1)

                    # score/exp chunks for the ib0 blocks (ib1's exps are
                    # produced post-loop: they bridge ScalarE across the
                    # quad boundary while AV backlogs and the next quad's
                    # projections occupy the PE). Emitted BEFORE the V
                    # groups so the first exps aren't queued behind them.
                    for tl in range(2):
                        ets[(0, tl)] += attn_qk(
                            kT_q, qT_q, tl, 0, range(4 * tb, 4 * tb + 4)
                        )
                    if q == 0 and tb >= 1:
                        # quad 0 has no previous quad bridging its token
                        # loop: add (ib1, tl0) chunks (lag 1) so ScalarE
                        # keeps pace with the PE-bound loop
                        ets[(1, 0)] += attn_qk(
                            kT_q, qT_q, 0, 1, range(4 * (tb - 1), 4 * tb)
                        )
                    if q < 2:
                        v_groups()
                    # drain (ib0, tl0) AVs lagging 2 key-tiles: their exps
                    # are long done so the PE never stalls on a fresh ACT
                    # (emitted after this tb's V writes — program order
                    # defines RAW semantics on v_all)
                    attn_av(q, 0, pos0, ets[(0, 0)][:-2])
                    ets[(0, 0)] = ets[(0, 0)][-2:]

                # ---- finish: sequential blocks, one po pair at a time ----
                # (ib1, tl0) exps first: they bridge ScalarE over the AV
                # backlog and the next quad's projections
                ets[(1, 0)] += attn_qk(
                    kT_q, qT_q, 0, 1, range(4 * len(ets[(1, 0)]) // 4, JT)
                )
                attn_av(q, 0, pos0, ets[(0, 0)])
                ets[(0, 0)] = []
                norm_tl(q, 0, 0, pos0)
                # (ib0, tl1): exps already buffered
                pos = (
                    opsum.tile([P, 512], f32, tag="po", name="po0"),
                    opsum.tile([P, 512], f32, tag="po", name="po1"),
                )
                attn_av(q, 1, pos, ets[(0, 1)])
                norm_tl(q, 1, 0, pos)
                if q == 2:
                    pre01 = [(g, final_git_pre(g)) for g in (0, 1)]
                    for g, pps in pre01:
                        final_git_post(g, pps)
                    for g in (2, 3):
                        final_git_post(g, final_git_pre(g))
                # ib1: score/exp emitted now — it keeps ScalarE busy across
                # the quad boundary while the PE runs AV backlogs and the
                # next quad's projections
                pre45 = None
                ets[(1, 1)] += attn_qk(kT_q, qT_q, 1, 1, range(0, 8))
                for tl in range(2):
                    if tl == 1:
                        ets[(1, 1)] += attn_qk(kT_q, qT_q, 1, 1, range(8, JT))
                    pos = (
                        opsum.tile([P, 512], f32, tag="po", name="po0"),
                        opsum.tile([P, 512], f32, tag="po", name="po1"),
                    )
                    attn_av(q, tl, pos, ets[(1, tl)])
                    norm_tl(q, tl, 1, pos)
                    if q == 2 and tl == 0:
                        pre45 = [(g, final_git_pre(g)) for g in (4, 5)]
                if q == 2:
                    for g, pps in pre45:
                        final_git_post(g, pps, tail=True)
                    for g in (6, 7):
                        final_git_post(g, final_git_pre(g), tail=True)

    nc.compile()
    return nc


def _get_nc():
    if "nc" not in _cache:
        _cache["nc"] = _build_bass()
    return _cache["nc"]


def _prep_in_maps(x, w_qkv, w_proj, b_proj):
    x = np.asarray(x, np.float32)
    w_qkv = np.asarray(w_qkv, np.float32)
    w_proj = np.asarray(w_proj, np.float32)
    b_proj = np.asarray(b_proj, np.float32)

    bf = ml_dtypes.bfloat16
    wq = np.ascontiguousarray(w_qkv[0:C].T).astype(bf)
    wk = np.ascontiguousarray(w_qkv[C : 2 * C].T).astype(bf)
    wv = np.ascontiguousarray(w_qkv[2 * C : 3 * C].T).astype(bf)
    wp = np.ascontiguousarray(w_proj.T).astype(bf)
    bb = np.ascontiguousarray(np.broadcast_to(b_proj[None, :], (P, C)))

    in_maps = []
    for core in range(NCORES):
        b, half = core // 2, core % 2
        xT = x[b].T  # [C, N]
        mine = xT[:, half * IQ : (half + 1) * IQ]
        other = xT[:, (1 - half) * IQ : (2 - half) * IQ]
        xt2 = np.concatenate([mine, other], axis=1)  # [C, N]
        # tiled [tb, p, o, n]: 6KB-contiguous per (tb, partition) DMA runs
        xt = np.ascontiguousarray(
            xt2.reshape(CT, P, N // TKB, TKB).transpose(2, 1, 0, 3)
        ).astype(bf)
        in_maps.append(
            {"xt": xt, "wq": wq, "wk": wk, "wv": wv, "wp": wp, "bb": bb}
        )
    return in_maps


def run(x, w_qkv, w_proj, b_proj, trace=False):
    from concourse import bass_utils

    nc = _get_nc()
    in_maps = _prep_in_maps(x, w_qkv, w_proj, b_proj)
    br = bass_utils.run_bass_kernel_spmd(
        nc, in_maps, core_ids=list(range(NCORES)), trace=trace
    )
    y = np.empty((B, N, C), np.float32)
    for core in range(NCORES):
        b, half = core // 2, core % 2
        y[b, half * IQ : (half + 1) * IQ, :] = br.results[core]["out"]
    return y, br


def kernel(x, w_qkv, w_proj, b_proj):
    y, _ = run(x, w_qkv, w_proj, b_proj, trace=False)
    return y


# revision 20
# speedup vs baseline: 1.0585x; 1.0016x over previous
"""Trainium2 Bass kernel for multi-head attention (B=4, N=2048, C=768, H=12).

Sharding: 8 cores = 4 batches x 2 sequence-halves. Each core computes K/V for
its batch's full 2048-token sequence (duplicated across the 2 cores sharing a
batch) and Q/attention/proj for its own 1024 query rows. No collectives; the
host gather is pure concatenation. The host passes x[b].T with the core's own
half rolled to the front, so Q-projection always reads columns 0:1024
(attention is permutation-invariant along keys, so rolling K/V is harmless).

v6: all-bf16 datapath (PSUM and the exp input stay fp32). bf16 stationary
operands get separate LDWEIGHTS, so the two 64-deep QK matmuls of a head pair
run concurrently as PE row tiles (0,0)/(64,0). V tiles are 65 columns (64 hd
+ ones row producing the softmax denominator in PSUM), so no memzero is
needed. ScalarE exp (25.2M elems/core at 1 elem/cyc/lane, ~213us busy) is the
pacing engine. The key structure: the QK+exp stream is DECOUPLED from the
AV/PSUM-accumulator constraint by buffering exp tiles in SBUF — every quad
emits score+exp chunks per token block as soon as K/Q land (attention starts
~8us in, and each quad's exps are ready during the previous quad's attention,
so ScalarE never starves at quad boundaries). AV matmuls trail, consuming
buffered exp tiles into the 2 live PSUM accumulator pairs. Normalization is
per head-pair (denominators packed on partitions 0-1 via tiny DMAs, one DVE
reciprocal, GpSimd partition_broadcast from partition 0, DVE multiply), and
the final projection pre-accumulates head pairs 0-4 during quad-2 attention
so only pair 5 + bias trail the last normalize. Startup DMAs are split
across the Sync and Activation HWDGE queues.
"""

import os
import ml_dtypes
import numpy as np

B, N, C = 4, 2048, 768
H, HD = 12, 64
SCALE = HD ** -0.5
P = 128
CT = C // P          # 6 contraction tiles
PAIRS = H // 2       # 6 head pairs
QUADS = H // 4       # 3 head quads
IQ = N // 2          # 1024 query rows per core
JT = N // P          # 16 key tiles
TKB = 512            # token-block width streamed from DRAM
VW = 72              # per-head stride in v_all (65 used: 64 hd + ones)
NCORES = 8

_cache = {}


def _build_bass():
    import concourse.bass as bass
    import concourse.tile as tile
    import concourse.mybir as mybir
    from concourse import bacc
    from concourse.bass import ts, ds
    from contextlib import ExitStack

    f32 = mybir.dt.float32
    bf16 = mybir.dt.bfloat16
    Exp = mybir.ActivationFunctionType.Exp

    nc = bacc.Bacc("TRN2", target_bir_lowering=False, debug=False)

    xt_d = nc.dram_tensor(
        "xt", [N // TKB, P, CT, TKB], bf16, kind="ExternalInput"
    ).ap()
    wq_d = nc.dram_tensor("wq", [C, C], bf16, kind="ExternalInput").ap()
    wk_d = nc.dram_tensor("wk", [C, C], bf16, kind="ExternalInput").ap()
    wv_d = nc.dram_tensor("wv", [C, C], bf16, kind="ExternalInput").ap()
    wp_d = nc.dram_tensor("wp", [C, C], bf16, kind="ExternalInput").ap()
    bb_d = nc.dram_tensor("bb", [P, C], f32, kind="ExternalInput").ap()
    out_d = nc.dram_tensor("out", [IQ, C], f32, kind="ExternalOutput").ap()

    wq_r = wq_d.rearrange("(o p) n -> p o n", p=P)
    wk_r = wk_d.rearrange("(o p) n -> p o n", p=P)
    wv_r = wv_d.rearrange("(o p) n -> p o n", p=P)
    wp_r = wp_d.rearrange("(o p) n -> p o n", p=P)
    out_r = out_d.rearrange("(t p) n -> t p n", p=P)

    with tile.TileContext(nc) as tc:
        with ExitStack() as ctx:
            persist = ctx.enter_context(tc.tile_pool(name="persist", bufs=1))
            outT_sb = persist.tile([P, PAIRS, IQ], bf16, name="outT_sb")
            v_all = persist.tile([P, JT, H * VW], bf16, name="v_all")
            v_all_r = v_all.rearrange("p t (h e) -> p t h e", e=VW)
            with nc.allow_low_precision(reason="ones column"):
                nc.vector.tensor_copy(
                    v_all_r[:, :, :, 64],
                    nc.const_aps.tensor(1.0, [P, JT, H], bf16),
                )

            wpool = ctx.enter_context(tc.tile_pool(name="wq", bufs=2))
            wvpool = ctx.enter_context(tc.tile_pool(name="wv", bufs=1))
            kvq = ctx.enter_context(tc.tile_pool(name="kvq", bufs=2))
            xt_pool = ctx.enter_context(tc.tile_pool(name="xtp", bufs=2))
            apsum = ctx.enter_context(
                tc.tile_pool(name="apsum", bufs=2, space="PSUM")
            )
            spsum = ctx.enter_context(
                tc.tile_pool(name="spsum", bufs=2, space="PSUM")
            )
            opsum = ctx.enter_context(
                tc.tile_pool(name="opsum", bufs=2, space="PSUM")
            )
            # deep exp-tile buffer: lets the QK+exp stream run far ahead of
            # the AV consumers (ib1 blocks' exps are fully buffered)
            expt_pool = ctx.enter_context(tc.tile_pool(name="expt", bufs=32))
            nrm_pool = ctx.enter_context(tc.tile_pool(name="nrm", bufs=2))
            poS_pool = ctx.enter_context(tc.tile_pool(name="poSp", bufs=4))
            ppool = ctx.enter_context(tc.tile_pool(name="pw", bufs=1))
            outsb_pool = ctx.enter_context(tc.tile_pool(name="outsb", bufs=2))

            wp_sb = None
            bias_sb = None

            def attn_qk(kT_q, qT_q, tl, ib, jts):
                ets = []
                for jt in jts:
                    ss = spsum.tile([P, 1024], f32, tag="ss", name="ss")
                    nc.tensor.matmul(
                        ss[:, 0:512],
                        kT_q[0:64, tl, ts(jt, P)],
                        qT_q[0:64, tl, ts(ib, 512)],
                        start=True,
                        stop=True,
                    )
                    nc.tensor.matmul(
                        ss[:, 512:1024],
                        kT_q[64:128, tl, ts(jt, P)],
                        qT_q[64:128, tl, ts(ib, 512)],
                        start=True,
                        stop=True,
                    )
                    et = expt_pool.tile([P, 1024], bf16, tag="et", name="et")
                    nc.scalar.activation(et[:], ss[:], Exp, scale=SCALE)
                    ets.append((jt, et))
                return ets

            def attn_av(q, tl, pos, ets):
                t = 2 * q + tl
                for jt, et in ets:
                    for hh in range(2):
                        hg = 2 * t + hh
                        nc.tensor.matmul(
                            pos[hh][0:65, :],
                            v_all_r[:, jt, hg, 0:65],
                            et[:, hh * 512 : (hh + 1) * 512],
                            start=(jt == 0),
                            stop=(jt == JT - 1),
                        )

            def norm_tl(q, tl, ib, pos):
                """Per-pair softmax normalization: outT = po[0:64] / po[64]."""
                t = 2 * q + tl
                dpk = nrm_pool.tile([2, 512], f32, tag="dpk", name="dpk")
                poSs = []
                for hh in range(2):
                    poS = poS_pool.tile([65, 512], f32, tag="poS", name="poS")
                    nc.vector.tensor_copy(poS[:], pos[hh][0:65, :])
                    nc.sync.dma_start(dpk[hh : hh + 1, :], poS[64:65, :])
                    poSs.append(poS)
                rd_q = nrm_pool.tile([2, 512], f32, tag="rd_q", name="rd_q")
                nc.vector.reciprocal(rd_q[:], dpk[:])
                for hh in range(2):
                    if hh == 0:
                        rd_src = rd_q
                    else:
                        # relocate to partition 0: HW partition_broadcast
                        # only sources partition 0 correctly
                        rd_src = nrm_pool.tile([1, 512], f32, tag="rd1", name="rd1")
                        nc.sync.dma_start(rd_src[:], rd_q[1:2, :])
                    rb_sb = nrm_pool.tile([64, 512], f32, tag="rb_sb", name="rb_sb")
                    nc.gpsimd.partition_broadcast(rb_sb[:], rd_src[0:1, :])
                    with nc.allow_low_precision(reason="bf16 out path"):
                        nc.vector.tensor_mul(
                            outT_sb[hh * 64 : (hh + 1) * 64, t, ts(ib, 512)],
                            poSs[hh][0:64, :],
                            rb_sb[:],
                        )

            def final_git_pre(git):
                """Accumulate head pairs 0..4 of the output projection."""
                pps = []
                for n0, n1 in ((0, 512), (512, 768)):
                    pp = apsum.tile([P, 512], f32, tag="aps", name="pp")
                    for t in range(PAIRS - 1):
                        nc.tensor.matmul(
                            pp[:, 0 : n1 - n0],
                            outT_sb[:, t, ds(git * P, P)],
                            wp_sb[:, t, n0:n1],
                            start=(t == 0),
                            stop=False,
                        )
                    pps.append(pp)
                return pps

            def final_git_post(git, pps, tail=False):
                """Last head pair + bias. The out DMA rides the Activation
                HWDGE queue only in the tail (after the last exp) — earlier it
                would block the ACT instruction stream."""
                ob = outsb_pool.tile([P, C], f32, tag="ob", name="ob")
                for (n0, n1), pp in zip(((0, 512), (512, 768)), pps):
                    nc.tensor.matmul(
                        pp[:, 0 : n1 - n0],
                        outT_sb[:, PAIRS - 1, ds(git * P, P)],
                        wp_sb[:, PAIRS - 1, n0:n1],
                        start=False,
                        stop=True,
                    )
                    nc.vector.tensor_add(
                        ob[:, n0:n1], pp[:, 0 : n1 - n0], bias_sb[:, n0:n1]
                    )
                (nc.scalar if tail else nc.sync).dma_start(out_r[git], ob[:])

            for q in range(QUADS):
                # ---- load this quad's weight slices ----
                wk_t = wpool.tile([P, CT, 256], bf16, tag="wk_t")
                wq_t = wpool.tile([P, CT, 256], bf16, tag="wq_t")
                if q == 0:
                    # parallelize the cold-start loads: wk/wq on the
                    # Activation queue, xt/wv on Sync
                    nc.scalar.dma_start(wk_t[:], wk_r[:, :, ts(q, 256)])
                    nc.scalar.dma_start(wq_t[:], wq_r[:, :, ts(q, 256)])
                else:
                    nc.sync.dma_start(wk_t[:], wk_r[:, :, ts(q, 256)])
                    nc.sync.dma_start(wq_t[:], wq_r[:, :, ts(q, 256)])
                if q == 0:
                    wv_t = wvpool.tile([P, CT, 512], bf16, tag="wv_t", name="wv_t")
                    nc.scalar.dma_start(wv_t[:], wv_r[:, :, 0:512])
                elif q == 1:
                    wv_t = wvpool.tile([P, CT, 256], bf16, tag="wv_t", name="wv_t")
                    nc.sync.dma_start(wv_t[:], wv_r[:, :, 512:768])
                if q == 2:
                    # stage final-projection weights during quad 2
                    wp_sb = ppool.tile([P, CT, C], bf16, name="wp_sb")
                    nc.sync.dma_start(wp_sb[:], wp_r)
                    bias_sb = ppool.tile([P, C], f32, name="bias_sb")
                    nc.sync.dma_start(bias_sb[:], bb_d)

                kT_q = kvq.tile([P, 2, N], bf16, tag="kT_q")
                qT_q = kvq.tile([P, 2, IQ], bf16, tag="qT_q")

                # exp-tile queues; PSUM accumulator pairs for the ib0 blocks
                # (their AVs drain inside the tb loop, after each V write)
                ets = {(ib, tl): [] for ib in range(2) for tl in range(2)}
                # only ONE po pair exists at a time (opsum = 2 slots), so
                # exactly one block accumulates in-loop: (ib0, tl0)
                pos0 = (
                    opsum.tile([P, 512], f32, tag="po", name="po0"),
                    opsum.tile([P, 512], f32, tag="po", name="po1"),
                )

                # ---- projections + score/exp chunks per token block ----
                for tb in range(N // TKB):
                    xt_t = xt_pool.tile([P, CT, TKB], bf16, tag="xt")
                    nc.sync.dma_start(xt_t[:], xt_d[tb])

                    def k_group(tl):
                        ps = apsum.tile([P, TKB], f32, tag="aps", name="ps")
                        for c in range(CT):
                            nc.tensor.matmul(
                                ps[:],
                                wk_t[:, c, ts(tl, P)],
                                xt_t[:, c, :],
                                start=(c == 0),
                                stop=(c == CT - 1),
                            )
                        with nc.allow_low_precision(reason="bf16 k path"):
                            nc.vector.tensor_copy(kT_q[:, tl, ts(tb, TKB)], ps[:])

                    def q_group(tl):
                        ps = apsum.tile([P, TKB], f32, tag="aps", name="ps")
                        for c in range(CT):
                            nc.tensor.matmul(
                                ps[:],
                                wq_t[:, c, ts(tl, P)],
                                xt_t[:, c, :],
                                start=(c == 0),
                                stop=(c == CT - 1),
                            )
                        with nc.allow_low_precision(reason="bf16 q path"):
                            nc.vector.tensor_copy(qT_q[:, tl, ts(tb, TKB)], ps[:])

                    def v_groups():
                        vn = 512 if q == 0 else 256
                        h0 = 0 if q == 0 else 8
                        for tt in range(TKB // P):
                            ps = apsum.tile([P, vn], f32, tag="aps", name="ps")
                            for c in range(CT):
                                nc.tensor.matmul(
                                    ps[:],
                                    xt_t[:, c, ts(tt, P)],
                                    wv_t[:, c, 0:vn],
                                    start=(c == 0),
                                    stop=(c == CT - 1),
                                )
                            gtt = (tb * TKB) // P + tt
                            with nc.allow_low_precision(reason="bf16 v path"):
                                nc.vector.tensor_copy(
                                    v_all_r[:, gtt, h0 : h0 + vn // 64, 0:64],
                                    ps.rearrange("p (h e) -> p h e", e=64),
                                )

                    if q > 0 and tb == 0:
                        # the first token block's K/Q must beat the previous
                        # quad's AV backlog to the PE, or this quad's first
                        # exps arrive ~7us after the old ACT stream dries
                        with tc.high_priority():
                            k_group(0)
                            k_group(1)
                            q_group(0)
                            q_group(1)
                    else:
                        k_group(0)
                        k_group(1)
                        if tb < IQ // TKB:
                            q_group(0)
                            q_group(1)

                    # score/exp chunks for the ib0 blocks (ib1's exps are
                    # produced post-loop: they bridge ScalarE across the
                    # quad boundary while AV backlogs and the next quad's
                    # projections occupy the PE). Emitted BEFORE the V
                    # groups so the first exps aren't queued behind them.
                    for tl in range(2):
                        ets[(0, tl)] += attn_qk(
                            kT_q, qT_q, tl, 0, range(4 * tb, 4 * tb + 4)
                        )
                    if q == 0 and tb >= 1:
                        # quad 0 has no previous quad bridging its token
                        # loop: add (ib1, tl0) chunks (lag 1) so ScalarE
                        # keeps pace with the PE-bound loop
                        ets[(1, 0)] += attn_qk(
                            kT_q, qT_q, 0, 1, range(4 * (tb - 1), 4 * tb)
                        )
                    if q < 2:
                        v_groups()
                    # drain (ib0, tl0) AVs lagging 2 key-tiles: their exps
                    # are long done so the PE never stalls on a fresh ACT
                    # (emitted after this tb's V writes — program order
                    # defines RAW semantics on v_all)
                    attn_av(q, 0, pos0, ets[(0, 0)][:-2])
                    ets[(0, 0)] = ets[(0, 0)][-2:]

                # ---- finish: sequential blocks, one po pair at a time ----
                # (ib1, tl0) exps first: they bridge ScalarE over the AV
                # backlog and the next quad's projections
                ets[(1, 0)] += attn_qk(
                    kT_q, qT_q, 0, 1, range(4 * len(ets[(1, 0)]) // 4, JT)
                )
                attn_av(q, 0, pos0, ets[(0, 0)])
                ets[(0, 0)] = []
                norm_tl(q, 0, 0, pos0)
                # (ib0, tl1): exps already buffered
                pos = (
                    opsum.tile([P, 512], f32, tag="po", name="po0"),
                    opsum.tile([P, 512], f32, tag="po", name="po1"),
                )
                attn_av(q, 1, pos, ets[(0, 1)])
                norm_tl(q, 1, 0, pos)
                if q == 2:
                    pre01 = [(g, final_git_pre(g)) for g in (0, 1)]
                    for g, pps in pre01:
                        final_git_post(g, pps)
                    for g in (2, 3):
                        final_git_post(g, final_git_pre(g))
                # ib1: score/exp emitted now — it keeps ScalarE busy across
                # the quad boundary while the PE runs AV backlogs and the
                # next quad's projections
                pre45 = None
                ets[(1, 1)] += attn_qk(kT_q, qT_q, 1, 1, range(0, 8))
                for tl in range(2):
                    if tl == 1:
                        ets[(1, 1)] += attn_qk(kT_q, qT_q, 1, 1, range(8, JT))
                    pos = (
                        opsum.tile([P, 512], f32, tag="po", name="po0"),
                        opsum.tile([P, 512], f32, tag="po", name="po1"),
                    )
                    attn_av(q, tl, pos, ets[(1, tl)])
                    norm_tl(q, tl, 1, pos)
                    if q == 2 and tl == 0:
                        pre45 = [(g, final_git_pre(g)) for g in (4, 5)]
                if q == 2:
                    for g, pps in pre45:
                        final_git_post(g, pps, tail=True)
                    for g in (6, 7):
                        final_git_post(g, final_git_pre(g), tail=True)

    nc.compile()
    return nc


def _get_nc():
    if "nc" not in _cache:
        _cache["nc"] = _build_bass()
    return _cache["nc"]


def _prep_in_maps(x, w_qkv, w_proj, b_proj):
    x = np.asarray(x, np.float32)
    w_qkv = np.asarray(w_qkv, np.float32)
    w_proj = np.asarray(w_proj, np.float32)
    b_proj = np.asarray(b_proj, np.float32)

    bf = ml_dtypes.bfloat16
    wq = np.ascontiguousarray(w_qkv[0:C].T).astype(bf)
    wk = np.ascontiguousarray(w_qkv[C : 2 * C].T).astype(bf)
    wv = np.ascontiguousarray(w_qkv[2 * C : 3 * C].T).astype(bf)
    wp = np.ascontiguousarray(w_proj.T).astype(bf)
    bb = np.ascontiguousarray(np.broadcast_to(b_proj[None, :], (P, C)))

    in_maps = []
    for core in range(NCORES):
        b, half = core // 2, core % 2
        xT = x[b].T  # [C, N]
        mine = xT[:, half * IQ : (half + 1) * IQ]
        other = xT[:, (1 - half) * IQ : (2 - half) * IQ]
        xt2 = np.concatenate([mine, other], axis=1)  # [C, N]
        # tiled [tb, p, o, n]: 6KB-contiguous per (tb, partition) DMA runs
        xt = np.ascontiguousarray(
            xt2.reshape(CT, P, N // TKB, TKB).transpose(2, 1, 0, 3)
        ).astype(bf)
        in_maps.append(
            {"xt": xt, "wq": wq, "wk": wk, "wv": wv, "wp": wp, "bb": bb}
        )
    return in_maps


def run(x, w_qkv, w_proj, b_proj, trace=False):
    from concourse import bass_utils

    nc = _get_nc()
    in_maps = _prep_in_maps(x, w_qkv, w_proj, b_proj)
    br = bass_utils.run_bass_kernel_spmd(
        nc, in_maps, core_ids=list(range(NCORES)), trace=trace
    )
    y = np.empty((B, N, C), np.float32)
    for core in range(NCORES):
        b, half = core // 2, core % 2
        y[b, half * IQ : (half + 1) * IQ, :] = br.results[core]["out"]
    return y, br


def kernel(x, w_qkv, w_proj, b_proj):
    y, _ = run(x, w_qkv, w_proj, b_proj, trace=False)
    return y
